# revision 18
# baseline (speedup 1.0000x reference)
"""Trainium2 Bass kernel for nn_MinimalRNNCell.

Reference math (fp32):
    z_t = W_in x_t + b_in
    u_t = sigmoid(Wg_h h_{t-1} + Wg_z z_t + b_g)
    h_t = u_t * h_{t-1} + (1-u_t) * z_t
    y_t = W_out h_t + b_out
    output = y[:, batch=-1, :]  -> [T, O]   (only batch element 63 matters!)

Strategy (Picard iteration on the gated recurrence):
  * Only sample 63 of the batch affects the output -> compute just that one.
  * Substitute m = h - z:  m_t = (Delta_t + m_{t-1}) * u_t with
    Delta_t = z_{t-1} - z_t.  GIVEN the gates u, this linear recurrence is
    solved over a whole 272/256-column chunk by a single DVE
    tensor_tensor_scan instruction (op0=add, op1=mult, fp32 carry, fp16
    out) reading Delta straight from PSUM.
  * The gates couple back through pre_t = Wg_h m_{t-1} + P2_t, where
    P2_t = Wg_z z_t + Wg_h z_{t-1} + b_g is m-independent.  Picard-iterate:
    m^0 = 0; each sweep recomputes u = sigmoid(P2 + Wg_h m^{k-1}) in bulk
    (one accumulating matmul + one activation per chunk) and re-runs the
    scan.  3 sweeps reach ~1e-3 rel err (gate is 2e-2).
  * 8 cores each own 512 contiguous timesteps plus W=16 warmup columns that
    absorb the unknown cross-core starting state (decay ~prod(u) ~ 0.5^16).
  * Only x is shipped (64 rows); the difference columns xd_t = x_{t-1}-x_t
    are derived on-chip by one DVE subtract.  P2 = (Wg_z+Wg_h)W_in x_t +
    Wg_h W_in xd_t via two 64-row matmuls into one accumulation group;
    weight biases ride the sigmoid bias operand.  Delta = W_in xd.
  * The output needs no h at all:  y_t = W_out m_t + (W_out W_in) x_t +
    (W_out b_in + b_out), so each 128-row output block is two matmuls
    (m16^T W_out + x^T folded) and the constant is added on the host.
  * The t=0 boundary of core 0 is slightly inexact (the b_in terms of the
    ghost column z_{-1}); the first HOST_ROWS outputs are recomputed
    exactly on the host and the on-chip residual decays ~0.5^t away.
  * A stream of junk matmuls at kernel start holds the PE busy so the HAM
    fast-clock ramp completes while the input DMAs fly; a junk sigmoid
    preloads the ACT table.
"""

import numpy as np

import concourse.bass as bass
import concourse.mybir as mybir
import concourse.tile as tile
from concourse import bacc
from concourse.bass_utils import run_bass_kernel_spmd

# problem constants (hardcoded per harness contract)
T, I, H, O = 4096, 64, 128, 64
NCORES = 8
TLOC = T // NCORES          # timesteps per core
W = 16                      # warmup columns absorbing the chunk boundary
NZ = 1 + W + TLOC           # columns per core (1 leading col for the shift)
NSW = 2                     # Picard sweeps (rel err ~9e-3 vs the 2e-2 gate)
C1 = 272                    # chunk-1 columns (cols 1..273)
C2 = NZ - 1 - C1            # 256: chunk-2 columns (cols 273..529)
BD = 1 + C1                 # 273: chunk boundary column
HOST_ROWS = 8               # exact host-computed leading output rows

# fp16 const blobs: c16lo [64, 384] = ((Wg_z+Wg_h)W_in)^T | (Wg_h W_in)^T |
# W_in^T; c16hi [128, 256] = Wg_h^T | W_out^T | (W_out W_in)^T (64 rows)
NCLO = 384
NCHI = 256
NC32 = 1                    # fp32 blob: sigmoid bias column

FP32 = mybir.dt.float32
FP16 = mybir.dt.float16
AF = mybir.ActivationFunctionType
ALU = mybir.AluOpType


def _build_program():
    nc = bacc.Bacc()

    x64 = nc.dram_tensor("x64", [I, NZ], FP16, kind="ExternalInput")
    c16lo = nc.dram_tensor("c16lo", [I, NCLO], FP16, kind="ExternalInput")
    c16hi = nc.dram_tensor("c16hi", [128, NCHI], FP16, kind="ExternalInput")
    c32 = nc.dram_tensor("c32", [128, NC32], FP32, kind="ExternalInput")
    y = nc.dram_tensor("y", [TLOC, O], FP32, kind="ExternalOutput")

    with tile.TileContext(nc) as tc:
        with (
            tc.tile_pool(name="singles", bufs=1) as singles,
            tc.tile_pool(name="upool", bufs=2) as upool,
            tc.tile_pool(name="psum_y", bufs=2, space="PSUM") as psum_y,
            tc.tile_pool(name="psum_d", bufs=2, space="PSUM") as psum_d,
            tc.tile_pool(name="psum_pre", bufs=3, space="PSUM") as psum_pre,
        ):
            # ---- PE clock-ramp stream + ACT sigmoid-table preload, both
            # overlapping the input DMAs ----
            junk = singles.tile([128, 512], FP16)
            nc.vector.memset(junk, 0.0)
            junk_sig = singles.tile([128, 1], FP32)
            ps_junk = psum_pre.tile([128, 512], FP32, tag="pre")
            for cols in (512, 512, 256, 256):
                nc.tensor.matmul(ps_junk[:, 0:cols], junk[:, 0:128],
                                 junk[:, 0:cols],
                                 start=True, stop=True, skip_group_check=True)

            # ---- input DMAs ----
            x64_sb = singles.tile([I, NZ], FP16)
            c16lo_sb = singles.tile([I, NCLO], FP16)
            c16hi_sb = singles.tile([128, NCHI], FP16)
            c32_sb = singles.tile([128, NC32], FP32)
            nc.sync.dma_start(out=x64_sb[:, 0:BD + 1], in_=x64[:, 0:BD + 1])
            nc.gpsimd.dma_start(out=x64_sb[:, BD + 1:NZ],
                                in_=x64[:, BD + 1:NZ])
            nc.scalar.dma_start(out=c16lo_sb, in_=c16lo[:, :])
            nc.sync.dma_start(out=c32_sb, in_=c32[:, :])
            nc.scalar.activation(junk_sig, junk[:, 0:1], AF.Sigmoid)
            nc.gpsimd.dma_start(out=c16hi_sb, in_=c16hi[:, :])

            abx_sb = c16lo_sb[:, 0:128]
            abd_sb = c16lo_sb[:, 128:256]
            win64_sb = c16lo_sb[:, 256:384]
            wghh_sb = c16hi_sb[:, 0:128]
            wout_sb = c16hi_sb[:, 128:192]
            wxo_sb = c16hi_sb[0:I, 192:256]
            bg_sb = c32_sb[:, 0:1]

            m16 = singles.tile([128, NZ], FP16)
            xdv = singles.tile([I, NZ], FP16)
            ysb = singles.tile([128, TLOC // 128, O], FP32)
            nc.vector.memset(m16[:, 0:1], 0.0)

            # xd_t = x_{t-1} - x_t derived on-chip (halved for earlier start)
            nc.vector.tensor_sub(xdv[:, 1:BD], x64_sb[:, 0:C1],
                                 x64_sb[:, 1:BD])
            nc.vector.tensor_sub(xdv[:, BD:NZ], x64_sb[:, BD - 1:NZ - 1],
                                 x64_sb[:, BD:NZ])

            dps = [psum_d.tile([128, 512], FP32, tag="d", name=f"d{c}")
                   for c in range(2)]
            pres = {}
            CHN = (C1, C2)

            def chunk_cols(c):
                lo = 1 + c * C1
                return lo, lo + CHN[c]

            def pre_mm(s, c):
                # the logical-priority gate keeps the scheduler from slotting
                # later sweeps' input-only matmuls ahead of the previous
                # sweep's critical accumulating matmul in the PE queue
                lo, hi = chunk_cols(c)
                n = CHN[c]
                ps = psum_pre.tile([128, 512], FP32, tag="pre", name=f"p{s}{c}")
                with tc.tile_wait_until(s * 1.0 + c * 0.1 if s > 0 else 0.3,
                                        enable=s > 0 or c > 0):
                    nc.tensor.matmul(ps[:, 0:n], abx_sb, x64_sb[:, lo:hi],
                                     start=True, stop=False)
                    nc.tensor.matmul(ps[:, 0:n], abd_sb, xdv[:, lo:hi],
                                     start=False, stop=(s == 0))
                    if s > 0:
                        nc.tensor.matmul(ps[:, 0:n], wghh_sb,
                                         m16[:, lo - 1:hi - 1],
                                         start=False, stop=True)
                pres[(s, c)] = ps

            def sigm(s, c):
                u = upool.tile([128, CHN[c]], FP16, tag="u", name=f"u{s}{c}")
                nc.scalar.activation(u, pres[(s, c)][:, 0:CHN[c]], AF.Sigmoid,
                                     bias=bg_sb)
                return u

            def scan(c, u):
                lo, hi = chunk_cols(c)
                init = 0.0 if c == 0 else m16[:, BD - 1:BD]
                nc.vector.tensor_tensor_scan(
                    m16[:, lo:hi], dps[c][:, 0:CHN[c]], u, init,
                    ALU.add, ALU.mult)

            # ---- pipeline, emitted in dataflow order ----
            pre_mm(0, 0)
            nc.tensor.matmul(dps[0][:, 0:C1], win64_sb, xdv[:, 1:BD],
                             start=True, stop=True)
            u = sigm(0, 0)
            scan(0, u)
            pre_mm(0, 1)
            with tc.tile_wait_until(0.3):
                nc.tensor.matmul(dps[1][:, 0:C2], win64_sb, xdv[:, BD:NZ],
                                 start=True, stop=True)
            u = sigm(0, 1)
            for s in range(1, NSW):
                pre_mm(s, 0)                   # reads m16 c1 <- prev c1 scan
                scan(1, u)
                u = sigm(s, 0)
                pre_mm(s, 1)
                scan(0, u)
                u = sigm(s, 1)

            # ---- y blocks: y = m16^T W_out + x^T (W_out W_in)^T ----
            def ymm(b):
                lo = W + 1 + b * 128
                ps_y = psum_y.tile([128, O], FP32, tag="y", name=f"y{b}")
                with tc.tile_wait_until(5.0 + b * 0.1):
                    nc.tensor.matmul(ps_y, x64_sb[:, lo:lo + 128], wxo_sb,
                                     start=True, stop=False)
                    nc.tensor.matmul(ps_y, m16[:, lo:lo + 128], wout_sb,
                                     start=False, stop=True)
                return ps_y

            yp0 = ymm(0)                       # blocks 0,1 only need scan c1
            yp1 = ymm(1)
            # final chunk-2 scan split in half so y2/y3 overlap it
            lo2 = BD + C2 // 2
            nc.vector.tensor_tensor_scan(
                m16[:, BD:lo2], dps[1][:, 0:C2 // 2], u[:, 0:C2 // 2],
                m16[:, BD - 1:BD], ALU.add, ALU.mult)
            yp2 = ymm(2)
            nc.vector.tensor_copy(ysb[:, 0, :], yp0)
            nc.vector.tensor_tensor_scan(
                m16[:, lo2:NZ], dps[1][:, C2 // 2:C2], u[:, C2 // 2:C2],
                m16[:, lo2 - 1:lo2], ALU.add, ALU.mult)
            y_view = y.rearrange("(b p) o -> p b o", p=128)
            nc.scalar.activation(ysb[:, 1, :], yp1, AF.Copy)
            nc.sync.dma_start(out=y_view[:, 0:2, :], in_=ysb[:, 0:2, :])
            yp3 = ymm(3)
            nc.vector.tensor_copy(ysb[:, 2, :], yp2)
            nc.vector.tensor_copy(ysb[:, 3, :], yp3)
            nc.sync.dma_start(out=y_view[:, 2:4, :], in_=ysb[:, 2:4, :])

    nc.compile()
    return nc


_PROGRAM = None


def _get_program():
    global _PROGRAM
    if _PROGRAM is None:
        _PROGRAM = _build_program()
    return _PROGRAM


def _prepare_in_maps(inputs):
    x = np.asarray(inputs["inputs"], dtype=np.float32)[63].astype(np.float64)
    W_in = np.asarray(inputs["W_in"], dtype=np.float64)
    b_g = np.asarray(inputs["b_g"], dtype=np.float64)
    b_in = np.asarray(inputs["b_in"], dtype=np.float64)
    W_g = np.asarray(inputs["W_g"], dtype=np.float64)
    W_out = np.asarray(inputs["W_out"], dtype=np.float64)
    Wg_h, Wg_z = W_g[:, :H], W_g[:, H:]

    c16lo = np.zeros((I, NCLO), np.float16)
    c16lo[:, 0:128] = ((Wg_z + Wg_h) @ W_in).T.astype(np.float16)
    c16lo[:, 128:256] = (Wg_h @ W_in).T.astype(np.float16)
    c16lo[:, 256:384] = W_in.T.astype(np.float16)
    c16hi = np.zeros((128, NCHI), np.float16)
    c16hi[:, 0:128] = Wg_h.T.astype(np.float16)
    c16hi[:, 128:192] = W_out.T.astype(np.float16)
    c16hi[0:I, 192:256] = (W_out @ W_in).T.astype(np.float16)

    c32 = np.zeros((128, NC32), np.float32)
    c32[:, 0] = (Wg_z + Wg_h) @ b_in + b_g

    xpad = np.concatenate([np.zeros((W + 1, I)), x], axis=0)
    in_maps = []
    for k in range(NCORES):
        lo = k * TLOC
        xk = np.ascontiguousarray(xpad[lo:lo + NZ].T.astype(np.float16))
        in_maps.append({"x64": xk, "c16lo": c16lo, "c16hi": c16hi,
                        "c32": c32})
    return in_maps


def _host_rows(inputs, K):
    """Exact (fp64) first K output rows; kills the t=0 boundary residual."""
    x = np.asarray(inputs["inputs"], dtype=np.float64)[63]
    W_in = np.asarray(inputs["W_in"], dtype=np.float64)
    b_in = np.asarray(inputs["b_in"], dtype=np.float64)
    W_g = np.asarray(inputs["W_g"], dtype=np.float64)
    b_g = np.asarray(inputs["b_g"], dtype=np.float64)
    W_out = np.asarray(inputs["W_out"], dtype=np.float64)
    b_out = np.asarray(inputs["b_out"], dtype=np.float64)
    Wg_h, Wg_z = W_g[:, :H], W_g[:, H:]
    h = np.zeros(H)
    out = np.zeros((K, O))
    for t in range(K):
        zt = W_in @ x[t] + b_in
        u = 1.0 / (1.0 + np.exp(-(Wg_h @ h + Wg_z @ zt + b_g)))
        h = u * h + (1.0 - u) * zt
        out[t] = W_out @ h + b_out
    return out.astype(np.float32)


def _y_const(inputs):
    W_in = np.asarray(inputs["W_in"], dtype=np.float64)
    b_in = np.asarray(inputs["b_in"], dtype=np.float64)
    W_out = np.asarray(inputs["W_out"], dtype=np.float64)
    b_out = np.asarray(inputs["b_out"], dtype=np.float64)
    return (W_out @ b_in + b_out).astype(np.float32)


def _run(in_maps, **kwargs):
    nc = _get_program()
    return run_bass_kernel_spmd(nc, in_maps, list(range(NCORES)), **kwargs)


def kernel(**inputs):
    res = _run(_prepare_in_maps(inputs))
    y = np.concatenate([res.results[k]["y"] for k in range(NCORES)], axis=0)
    y = y.astype(np.float32) + _y_const(inputs)[None, :]
    y[:HOST_ROWS] = _host_rows(inputs, HOST_ROWS)
    return np.ascontiguousarray(y)


if __name__ == "__main__":
    d = np.load("/root/problem/inputs.npz")
    out = kernel(**{k: d[k] for k in d.files})
    exp = np.load("/root/problem/expected.npy")
    err = np.abs(out - exp).max()
    print("absmax err vs expected:", err, " rel:", err / np.abs(exp).max())
